# revision 1
# baseline (speedup 1.0000x reference)
"""Trainium2 Bass kernel for the ContractiveREN problem.

Strategy
--------
Data parallel over the batch: each of the 8 NeuronCores gets a 2048-row
shard of ``u_in``; all (small) parameter matrices are folded on the host
into four 128x128 matmul weights plus two per-partition bias vectors.

Math
----
The reference computes (per batch row u, with x0 the initial state):
    w_i   = tanh((xc_i + ud_i + sum_{j<i} D11_ij w_j) / Lam_i)   (i = 0..127)
    y     = u @ Gu^T + w @ Gw^T + c0
where everything except the w-recurrence is affine in (u, w) and folds into
    Lhat = D11 / Lam[:,None],           UDb = (D12/Lam) @ u^T + xc/Lam
    Gu   = C2 @ inv(E) @ B2 + D22,      Gw = C2 @ inv(E) @ B1 + D21
    c0   = C2 @ inv(E) @ F @ x0
The strictly-lower-triangular recurrence is solved by fixed-point
iteration  W <- tanh(Lhat @ W + UDb)  which converges to fp32 precision in
~12 iterations (measured: absmax err 4.6e-8 at m=12; the tanh derivative
plus the rapid decay of ||Lhat^k|| gives ~3.7x error reduction per pass).
This keeps the whole kernel in matmul-friendly [feature, batch] layout:
no sequential 128-step scan, no per-step layout shuffles.

On-device pipeline (per core, batch shard 2048, all fp32):
  1. DMA u in 4 slabs, PE-transpose to Ut [128in, 2048b].
  2. UD = (D12/Lam)^T-matmul(Ut) in PSUM; W1 = tanh(UD + xc/Lam) via ACT
     bias; UDb = UD + xc/Lam via DVE tensor_scalar.
  3. M-1 Jacobi passes: PSUM = Lhat@W + I@UDb (two fp32r matmuls per
     512-batch chunk), ACT tanh -> next W.
  4. Yt = Gu@Ut + Gw@W + c0; PE-transpose back to batch-major; DMA out.
"""

import numpy as np

import concourse.bass as bass
import concourse.mybir as mybir
import concourse.tile as tile
from concourse import bacc
from concourse.bass_utils import run_bass_kernel_spmd

B = 16384
N_CORES = 8
BC = B // N_CORES  # 2048 batch rows per core
DIM_IN = 128
DIM_OUT = 128
DIM_X = 512
DIM_NL = 128
EPS = 1e-3
ALPHA = 1.0
M_FAST = 6   # Jacobi passes with float32r (e8m11) matmuls — 4x faster on PE
M_EXACT = 2  # final Jacobi passes with exact fp32 matmuls
# total tanh passes = 1 (seed) + M_FAST + M_EXACT; measured w abs err 7.3e-6
NCH = BC // 512  # batch chunks of 512 (PSUM bank / fp32 moving-operand limit)
NGR = BC // 512  # DMA slab groups (4 chunks of 128 rows each)
F32 = mybir.dt.float32
F32R = mybir.dt.float32r
TANH = mybir.ActivationFunctionType.Tanh

_BUILT = {}


def _round_f32r(x):
    """Round fp32 values to e8m11 (the float32r storage format)."""
    x = np.ascontiguousarray(x, np.float32)
    bits = x.view(np.uint32)
    out = ((bits + np.uint32(0x800)) & np.uint32(0xFFFFF000)).view(np.float32)
    return np.ascontiguousarray(out)


def _build_nc():
    nc = bacc.Bacc("TRN2", target_bir_lowering=False, debug=False)
    u = nc.dram_tensor("u", [BC, DIM_IN], F32, kind="ExternalInput").ap()
    cst = nc.dram_tensor("cst", [128, 642], F32, kind="ExternalInput").ap()
    # Lhat^T pre-rounded to e8m11 on the host, typed float32r for the
    # fast Jacobi matmuls (walrus requires fp32r matmul inputs to be
    # fp32r-rounded at the producer).
    ltr = nc.dram_tensor("ltr", [128, 128], F32R, kind="ExternalInput").ap()
    y = nc.dram_tensor("y", [BC, DIM_OUT], F32, kind="ExternalOutput").ap()

    # DRAM views: slab g holds chunks (rows) [g*512, (g+1)*512); within a
    # slab, SBUF partition p / sub-chunk k maps to DRAM row g*512 + k*128 + p.
    u_r = u.rearrange("(g k p) f -> g p k f", k=4, p=128)
    y_r = y.rearrange("(g k p) f -> g p k f", k=4, p=128)

    with tile.TileContext(nc) as tc:
        with (
            tc.tile_pool(name="const", bufs=1) as cpool,
            tc.tile_pool(name="big", bufs=1) as bpool,
            tc.tile_pool(name="w", bufs=3) as wpool,
            tc.tile_pool(name="stage", bufs=4) as spool,
            tc.tile_pool(name="ps", bufs=8, space="PSUM") as ppool,
        ):
            cst_t = cpool.tile([128, 642], F32)
            nc.sync.dma_start(cst_t[:], cst)
            ltr_t = cpool.tile([128, 128], F32R, tag="ltr")
            nc.sync.dma_start(ltr_t[:], ltr)
            lt = cst_t[:, 0:128]       # Lhat^T
            d12lt = cst_t[:, 128:256]  # (D12/Lam)^T
            gut = cst_t[:, 256:384]    # Gu^T
            gwt = cst_t[:, 384:512]    # Gw^T
            idt = cst_t[:, 512:640]    # identity
            xcl = cst_t[:, 640:641]    # xc/Lam  [128,1]
            c0 = cst_t[:, 641:642]     # C2 Einv F x0  [128,1]

            ut = bpool.tile([128, BC], F32, tag="ut")
            udb = bpool.tile([128, BC], F32, tag="udb")
            yt = bpool.tile([128, BC], F32, tag="yt")

            # ---- load u and transpose to feature-major Ut ----
            for g in range(NGR):
                ustage = spool.tile([128, 512], F32, tag="ustage")
                nc.sync.dma_start(
                    ustage[:].rearrange("p (k f) -> p k f", k=4), u_r[g]
                )
                pst = ppool.tile([128, 512], F32, tag="ps")
                for k in range(4):
                    ksl = slice(k * 128, (k + 1) * 128)
                    nc.tensor.transpose(pst[:, ksl], ustage[:, ksl], idt)
                sl = slice(g * 512, (g + 1) * 512)
                if g % 2 == 0:
                    nc.vector.tensor_copy(ut[:, sl], pst[:])
                else:
                    nc.scalar.copy(ut[:, sl], pst[:])

            # ---- seed: UD matmul, W1 = tanh(UD + xcl), UDb = UD + xcl ----
            # Per-chunk W tiles: keeps the 4 batch-chunk pipelines
            # independent in the Tile dependency graph, so pass m+1 of
            # chunk n overlaps pass m of chunk n+1.  W1 is float32r (ACT
            # rounds on write) for the fast fp32r passes.
            w_cur = [None] * NCH
            for n in range(NCH):
                sl = slice(n * 512, (n + 1) * 512)
                ps = ppool.tile([128, 512], F32, tag="ps")
                nc.tensor.matmul(ps[:], d12lt, ut[:, sl], start=True, stop=True)
                wt = wpool.tile([128, 512], F32R, tag=f"wr{n}")
                nc.scalar.activation(wt[:], ps[:], TANH, bias=xcl, scale=1.0)
                w_cur[n] = wt
                nc.vector.tensor_scalar_add(udb[:, sl], ps[:], xcl)

            # ---- fast Jacobi passes (fp32r matmuls, 1 cy/row) ----
            for _m in range(M_FAST):
                for n in range(NCH):
                    sl = slice(n * 512, (n + 1) * 512)
                    ps = ppool.tile([128, 512], F32, tag="ps")
                    nc.tensor.matmul(
                        ps[:], ltr_t[:], w_cur[n][:], start=True, stop=True
                    )
                    wt = wpool.tile([128, 512], F32R, tag=f"wr{n}")
                    nc.vector.tensor_add(ps[:], ps[:], udb[:, sl])
                    nc.scalar.activation(wt[:], ps[:], TANH)
                    w_cur[n] = wt

            # ---- exact fp32 Jacobi passes (polish off the fp32r floor) ----
            for _m in range(M_EXACT):
                for n in range(NCH):
                    sl = slice(n * 512, (n + 1) * 512)
                    ps = ppool.tile([128, 512], F32, tag="ps")
                    nc.tensor.matmul(
                        ps[:], lt, w_cur[n][:].bitcast(F32), start=True, stop=True
                    )
                    wt = wpool.tile([128, 512], F32, tag=f"w{n}")
                    nc.vector.tensor_add(ps[:], ps[:], udb[:, sl])
                    nc.scalar.activation(wt[:], ps[:], TANH)
                    w_cur[n] = wt

            # ---- output: Yt = Gu@Ut + Gw@W + c0 ----
            for n in range(NCH):
                sl = slice(n * 512, (n + 1) * 512)
                ps = ppool.tile([128, 512], F32, tag="ps")
                nc.tensor.matmul(ps[:], gut, ut[:, sl], start=True, stop=False)
                nc.tensor.matmul(ps[:], gwt, w_cur[n][:], start=False, stop=True)
                nc.vector.tensor_scalar_add(yt[:, sl], ps[:], c0)

            # ---- transpose back to batch-major and store ----
            for g in range(NGR):
                pst = ppool.tile([128, 512], F32, tag="ps")
                for k in range(4):
                    ksl = slice(k * 128, (k + 1) * 128)
                    csl = slice((g * 4 + k) * 128, (g * 4 + k + 1) * 128)
                    nc.tensor.transpose(pst[:, ksl], yt[:, csl], idt)
                ostage = spool.tile([128, 512], F32, tag="ostage")
                if g % 2 == 0:
                    nc.scalar.copy(ostage[:], pst[:])
                else:
                    nc.vector.tensor_copy(ostage[:], pst[:])
                nc.sync.dma_start(
                    y_r[g], ostage[:].rearrange("p (k f) -> p k f", k=4)
                )
    nc.compile()
    return nc


def _derive_host_params(X, Y, B2, C2, D21, D22, D12, x0):
    """Fold the contractive parameterization into kernel constants (fp32,
    mirroring the reference's fp32 op order as closely as practical)."""
    f = np.float32
    X = np.ascontiguousarray(X, f)
    H = (X.T @ X + EPS * np.eye(DIM_H, dtype=f)).astype(f)
    H11 = H[:DIM_X, :DIM_X]
    H21 = H[DIM_X:DIM_X + DIM_NL, :DIM_X]
    H22 = H[DIM_X:DIM_X + DIM_NL, DIM_X:DIM_X + DIM_NL]
    H31 = H[DIM_X + DIM_NL:, :DIM_X]
    H32 = H[DIM_X + DIM_NL:, DIM_X:DIM_X + DIM_NL]
    H33 = H[DIM_X + DIM_NL:, DIM_X + DIM_NL:]
    F = H31
    B1 = H32
    E = (0.5 * (H11 + ALPHA * H33 + Y - Y.T)).astype(f)
    Lam = (0.5 * np.diagonal(H22)).astype(f)
    D11 = (-np.tril(H22, k=-1)).astype(f)
    C1 = -H21

    Einv = np.linalg.inv(E).astype(f)
    x0v = np.asarray(x0, f)[0, 0, :]
    xc = (C1 @ x0v).astype(f)
    fx = (F @ x0v).astype(f)

    Lhat = (D11 / Lam[:, None]).astype(f)
    D12L = (np.asarray(D12, f) / Lam[:, None]).astype(f)
    CE = (np.asarray(C2, f) @ Einv).astype(f)
    Gu = (CE @ B2 + D22).astype(f)
    Gw = (CE @ B1 + D21).astype(f)
    xclam = (xc / Lam).astype(f)
    c0 = (CE @ fx).astype(f)

    cst = np.zeros((128, 642), f)
    cst[:, 0:128] = Lhat.T
    cst[:, 128:256] = D12L.T
    cst[:, 256:384] = Gu.T
    cst[:, 384:512] = Gw.T
    cst[:, 512:640] = np.eye(128, dtype=f)
    cst[:, 640] = xclam
    cst[:, 641] = c0
    return cst


DIM_H = 2 * DIM_X + DIM_NL


def kernel(u_in, X, Y, B2, C2, D21, D22, D12, x0):
    cst = _derive_host_params(X, Y, B2, C2, D21, D22, D12, x0)
    u = np.ascontiguousarray(np.asarray(u_in, np.float32).reshape(B, DIM_IN))

    if "nc" not in _BUILT:
        _BUILT["nc"] = _build_nc()
    nc = _BUILT["nc"]

    ltr = _round_f32r(cst[:, 0:128])
    in_maps = [
        {"u": u[i * BC:(i + 1) * BC], "cst": cst, "ltr": ltr}
        for i in range(N_CORES)
    ]
    res = run_bass_kernel_spmd(nc, in_maps, core_ids=list(range(N_CORES)))
    out = np.concatenate([res.results[i]["y"] for i in range(N_CORES)], axis=0)
    return out.reshape(B, 1, DIM_OUT).astype(np.float32)



# revision 5
# speedup vs baseline: 1.7493x; 1.7493x over previous
"""Trainium2 Bass kernel for the ContractiveREN problem.

Strategy
--------
Data parallel over the batch: each of the 8 NeuronCores gets a 2048-row
shard of ``u_in``; all (small) parameter matrices are folded on the host
into four 128x128 f32r matmul weights plus two per-partition bias vectors.

Math
----
The reference computes (per batch row u, with x0 the initial state):
    w_i   = tanh((xc_i + ud_i + sum_{j<i} D11_ij w_j) / Lam_i)   (i = 0..127)
    y     = u @ Gu^T + w @ Gw^T + c0
where everything except the w-recurrence is affine in (u, w) and folds into
    Lhat = D11 / Lam[:,None],           UD = (D12/Lam) @ u^T
    Gu   = C2 @ inv(E) @ B2 + D22,      Gw = C2 @ inv(E) @ B1 + D21
    c0   = C2 @ inv(E) @ F @ x0,        xclam = (C1 @ x0) / Lam
The strictly-lower-triangular recurrence is solved by fixed-point
iteration  W <- tanh(Lhat @ W + UD + xclam), contracting ~3.7x per pass.
With the 2e-2 correctness gate, TANH_TOTAL=4 passes suffice (measured
y_rel ~1.1e-3 vs the fp32 reference including f32r rounding effects).

On-device pipeline (per core, batch shard 2048, chunks of 512):
  1. DMA u in 4 slabs with 2KB-contiguous descriptors (batch rows
     interleaved 4-per-partition), PE-transpose to Ut [128in, 2048b],
     copy PSUM->SBUF as f32r (DVE/Pool).
  2. Seed: PSUM = (D12/Lam)^T-matmul(Ut) (f32r, 1cy/row); ACT tanh with
     bias=xclam -> W1 (f32r).
  3. 3 Jacobi passes: PSUM = Lhat@W + D12L@Ut (two accumulating f32r
     matmuls - no UDb tile, no DVE add), ACT tanh + bias -> next W.
  4. Yt = Gu@Ut + Gw@W (f32r); DVE adds c0; PE-transpose back to
     batch-major; copy PSUM->SBUF; DMA out (2KB descriptors).
"""

import numpy as np

import concourse.bass as bass
import concourse.mybir as mybir
import concourse.tile as tile
from concourse import bacc
from concourse.bass_utils import run_bass_kernel_spmd

B = 16384
N_CORES = 8
BC = B // N_CORES  # 2048 batch rows per core
DIM_IN = 128
DIM_OUT = 128
DIM_X = 512
DIM_NL = 128
DIM_H = 2 * DIM_X + DIM_NL
EPS = 1e-3
ALPHA = 1.0
N_PASS = 3  # Jacobi passes after the seed; tanh total = 1 + N_PASS
NCH = BC // 512  # batch chunks of 512 (PSUM bank size / slab granularity)
F32 = mybir.dt.float32
F32R = mybir.dt.float32r
TANH = mybir.ActivationFunctionType.Tanh

_BUILT = {}


def _round_f32r(x):
    """Round fp32 values to e8m11 (the float32r storage format)."""
    x = np.ascontiguousarray(x, np.float32)
    bits = x.view(np.uint32)
    out = ((bits + np.uint32(0x800)) & np.uint32(0xFFFFF000)).view(np.float32)
    return np.ascontiguousarray(out)


def _build_nc():
    nc = bacc.Bacc("TRN2", target_bir_lowering=False, debug=False)
    u = nc.dram_tensor("u", [BC, DIM_IN], F32, kind="ExternalInput").ap()
    # wts: four 128x128 stationary matrices, host-rounded to e8m11:
    #   [Lhat^T | (D12/Lam)^T | Gu^T | Gw^T]
    wts = nc.dram_tensor("wts", [128, 512], F32R, kind="ExternalInput").ap()
    # cst: fp32 identity (for PE transposes) + xclam + c0 column vectors
    cst = nc.dram_tensor("cst", [128, 130], F32, kind="ExternalInput").ap()
    y = nc.dram_tensor("y", [BC, DIM_OUT], F32, kind="ExternalOutput").ap()

    # Batch rows interleaved so every partition's slab rows are contiguous
    # in DRAM (2KB descriptors): slab g, partition p holds rows
    # 512g + 4p + r (r = 0..3); SBUF free index = r*128 + f.
    u_r = u.rearrange("(g p r) f -> g p (r f)", p=128, r=4)
    y_r = y.rearrange("(g p r) f -> g p (r f)", p=128, r=4)

    with tile.TileContext(nc) as tc:
        with (
            tc.tile_pool(name="const", bufs=1) as cpool,
            tc.tile_pool(name="big", bufs=1) as bpool,
            tc.tile_pool(name="w", bufs=3) as wpool,
            tc.tile_pool(name="stage", bufs=4) as spool,
            tc.tile_pool(name="ps", bufs=8, space="PSUM") as ppool,
        ):
            ut = bpool.tile([128, BC], F32R, tag="ut")
            yt = bpool.tile([128, BC], F32, tag="yt")
            cst_t = cpool.tile([128, 130], F32)
            wts_t = cpool.tile([128, 512], F32R, tag="wts")

            # ---- load u slab 0 first, then the small constants ----
            ustage = [
                spool.tile([128, 512], F32, tag="ustage", name=f"ustage{g}")
                for g in range(NCH)
            ]
            nc.sync.dma_start(ustage[0][:], u_r[0])
            nc.sync.dma_start(wts_t[:], wts)
            nc.sync.dma_start(cst_t[:], cst)
            for g in range(1, NCH):
                nc.sync.dma_start(ustage[g][:], u_r[g])

            idt = cst_t[:, 0:128]      # fp32 identity
            xcl = cst_t[:, 128:129]    # xc/Lam  [128,1]
            c0 = cst_t[:, 129:130]     # C2 Einv F x0  [128,1]
            lt = wts_t[:, 0:128]       # Lhat^T
            d12lt = wts_t[:, 128:256]  # (D12/Lam)^T
            gut = wts_t[:, 256:384]    # Gu^T
            gwt = wts_t[:, 384:512]    # Gw^T

            # ---- transpose u slabs to feature-major Ut (f32r) ----
            for g in range(NCH):
                pst = ppool.tile([128, 512], F32, tag="ps")
                for k in range(4):
                    ksl = slice(k * 128, (k + 1) * 128)
                    nc.tensor.transpose(pst[:, ksl], ustage[g][:, ksl], idt)
                sl = slice(g * 512, (g + 1) * 512)
                if g % 2 == 0:
                    nc.vector.tensor_copy(ut[:, sl], pst[:])
                else:
                    nc.scalar.copy(ut[:, sl], pst[:])

            # ---- seed: W1 = tanh(D12L@Ut + xclam) ----
            w_cur = [None] * NCH
            for n in range(NCH):
                sl = slice(n * 512, (n + 1) * 512)
                ps = ppool.tile([128, 512], F32, tag="ps")
                nc.tensor.matmul(ps[:], d12lt, ut[:, sl], start=True, stop=True)
                wt = wpool.tile([128, 512], F32R, tag=f"w{n}")
                nc.scalar.activation(wt[:], ps[:], TANH, bias=xcl)
                w_cur[n] = wt

            # ---- Jacobi passes: W <- tanh(Lhat@W + D12L@Ut + xclam) ----
            # The constant UD term is recomputed by a second accumulating
            # matmul (same PE cost as adding a stored UDb, but no DVE add
            # and no extra SBUF tile); xclam rides the ACT bias.
            for _m in range(N_PASS):
                for n in range(NCH):
                    sl = slice(n * 512, (n + 1) * 512)
                    ps = ppool.tile([128, 512], F32, tag="ps")
                    nc.tensor.matmul(
                        ps[:], lt, w_cur[n][:], start=True, stop=False
                    )
                    nc.tensor.matmul(
                        ps[:], d12lt, ut[:, sl], start=False, stop=True
                    )
                    wt = wpool.tile([128, 512], F32R, tag=f"w{n}")
                    nc.scalar.activation(wt[:], ps[:], TANH, bias=xcl)
                    w_cur[n] = wt

            # ---- output: Yt = Gu@Ut + Gw@W + c0 ----
            for n in range(NCH):
                sl = slice(n * 512, (n + 1) * 512)
                ps = ppool.tile([128, 512], F32, tag="ps")
                nc.tensor.matmul(ps[:], gut, ut[:, sl], start=True, stop=False)
                nc.tensor.matmul(ps[:], gwt, w_cur[n][:], start=False, stop=True)
                nc.vector.tensor_scalar_add(yt[:, sl], ps[:], c0)

            # ---- transpose back to batch-major and store ----
            for g in range(NCH):
                pst = ppool.tile([128, 512], F32, tag="ps")
                for k in range(4):
                    ksl = slice(k * 128, (k + 1) * 128)
                    csl = slice(g * 512 + k * 128, g * 512 + (k + 1) * 128)
                    nc.tensor.transpose(pst[:, ksl], yt[:, csl], idt)
                ostage = spool.tile([128, 512], F32, tag="ostage")
                if g % 2 == 0:
                    nc.scalar.copy(ostage[:], pst[:])
                else:
                    nc.vector.tensor_copy(ostage[:], pst[:])
                nc.sync.dma_start(y_r[g], ostage[:])
    nc.compile()
    return nc


def _derive_host_params(X, Y, B2, C2, D21, D22, D12, x0):
    """Fold the contractive parameterization into kernel constants (fp32,
    mirroring the reference's fp32 op order as closely as practical)."""
    f = np.float32
    X = np.ascontiguousarray(X, f)
    H = (X.T @ X + EPS * np.eye(DIM_H, dtype=f)).astype(f)
    H11 = H[:DIM_X, :DIM_X]
    H21 = H[DIM_X:DIM_X + DIM_NL, :DIM_X]
    H22 = H[DIM_X:DIM_X + DIM_NL, DIM_X:DIM_X + DIM_NL]
    H31 = H[DIM_X + DIM_NL:, :DIM_X]
    H32 = H[DIM_X + DIM_NL:, DIM_X:DIM_X + DIM_NL]
    H33 = H[DIM_X + DIM_NL:, DIM_X + DIM_NL:]
    F = H31
    B1 = H32
    E = (0.5 * (H11 + ALPHA * H33 + Y - Y.T)).astype(f)
    Lam = (0.5 * np.diagonal(H22)).astype(f)
    D11 = (-np.tril(H22, k=-1)).astype(f)
    C1 = -H21

    Einv = np.linalg.inv(E).astype(f)
    x0v = np.asarray(x0, f)[0, 0, :]
    xc = (C1 @ x0v).astype(f)
    fx = (F @ x0v).astype(f)

    Lhat = (D11 / Lam[:, None]).astype(f)
    D12L = (np.asarray(D12, f) / Lam[:, None]).astype(f)
    CE = (np.asarray(C2, f) @ Einv).astype(f)
    Gu = (CE @ B2 + D22).astype(f)
    Gw = (CE @ B1 + D21).astype(f)
    xclam = (xc / Lam).astype(f)
    c0 = (CE @ fx).astype(f)

    wts = np.zeros((128, 512), f)
    wts[:, 0:128] = Lhat.T
    wts[:, 128:256] = D12L.T
    wts[:, 256:384] = Gu.T
    wts[:, 384:512] = Gw.T
    wts = _round_f32r(wts)

    cst = np.zeros((128, 130), f)
    cst[:, 0:128] = np.eye(128, dtype=f)
    cst[:, 128] = xclam
    cst[:, 129] = c0
    return cst, wts


def _in_maps(u_in, X, Y, B2, C2, D21, D22, D12, x0):
    cst, wts = _derive_host_params(X, Y, B2, C2, D21, D22, D12, x0)
    u = np.ascontiguousarray(np.asarray(u_in, np.float32).reshape(B, DIM_IN))
    return [
        {"u": u[i * BC:(i + 1) * BC], "cst": cst, "wts": wts}
        for i in range(N_CORES)
    ]


def kernel(u_in, X, Y, B2, C2, D21, D22, D12, x0):
    in_maps = _in_maps(u_in, X, Y, B2, C2, D21, D22, D12, x0)
    if "nc" not in _BUILT:
        _BUILT["nc"] = _build_nc()
    nc = _BUILT["nc"]
    res = run_bass_kernel_spmd(nc, in_maps, core_ids=list(range(N_CORES)))
    out = np.concatenate([res.results[i]["y"] for i in range(N_CORES)], axis=0)
    return out.reshape(B, 1, DIM_OUT).astype(np.float32)


# revision 11
# speedup vs baseline: 1.8070x; 1.0330x over previous
"""Trainium2 Bass kernel for the ContractiveREN problem.

Strategy
--------
Data parallel over the batch: each of the 8 NeuronCores gets a 2048-row
shard of ``u_in``; all (small) parameter matrices are folded on the host
into four 128x128 f32r matmul weights plus two per-partition bias vectors.

Math
----
The reference computes (per batch row u, with x0 the initial state):
    w_i   = tanh((xc_i + ud_i + sum_{j<i} D11_ij w_j) / Lam_i)   (i = 0..127)
    y     = u @ Gu^T + w @ Gw^T + c0
where everything except the w-recurrence is affine in (u, w) and folds into
    Lhat = D11 / Lam[:,None],           UD = (D12/Lam) @ u^T
    Gu   = C2 @ inv(E) @ B2 + D22,      Gw = C2 @ inv(E) @ B1 + D21
    c0   = C2 @ inv(E) @ F @ x0,        xclam = (C1 @ x0) / Lam
The strictly-lower-triangular recurrence is solved by fixed-point
iteration  W <- tanh(Lhat @ W + UD + xclam), contracting ~3.7x per pass.
With the 2e-2 correctness gate, TANH_TOTAL=4 passes suffice (measured
y_rel ~1.1e-3 vs the fp32 reference including f32r rounding effects).

On-device pipeline (per core, batch shard 2048, chunks of 512):
  1. DMA u in 4 slabs with 2KB-contiguous descriptors (batch rows
     interleaved 4-per-partition), PE-transpose to Ut [128in, 2048b],
     copy PSUM->SBUF as f32r (DVE/Pool).
  2. Seed: PSUM = (D12/Lam)^T-matmul(Ut) (f32r, 1cy/row); ACT tanh with
     bias=xclam -> W1 (f32r).
  3. 3 Jacobi passes: PSUM = Lhat@W + D12L@Ut (two accumulating f32r
     matmuls - no UDb tile, no DVE add), ACT tanh + bias -> next W.
  4. Yt = Gu@Ut + Gw@W (f32r); DVE adds c0; PE-transpose back to
     batch-major; copy PSUM->SBUF; DMA out (2KB descriptors).
"""

import numpy as np

import concourse.bass as bass
import concourse.mybir as mybir
import concourse.tile as tile
from concourse import bacc
from concourse.bass_utils import run_bass_kernel_spmd

B = 16384
N_CORES = 8
BC = B // N_CORES  # 2048 batch rows per core
DIM_IN = 128
DIM_OUT = 128
DIM_X = 512
DIM_NL = 128
DIM_H = 2 * DIM_X + DIM_NL
EPS = 1e-3
ALPHA = 1.0
N_PASS = 2  # Jacobi passes after the seed; tanh total = 1 + N_PASS
NCH = BC // 512  # batch chunks of 512 (PSUM bank size / slab granularity)
F32 = mybir.dt.float32
F32R = mybir.dt.float32r
TANH = mybir.ActivationFunctionType.Tanh

_BUILT = {}


def _round_f32r(x):
    """Round fp32 values to e8m11 (the float32r storage format)."""
    x = np.ascontiguousarray(x, np.float32)
    bits = x.view(np.uint32)
    out = ((bits + np.uint32(0x800)) & np.uint32(0xFFFFF000)).view(np.float32)
    return np.ascontiguousarray(out)


def _build_nc():
    nc = bacc.Bacc("TRN2", target_bir_lowering=False, debug=False)
    u = nc.dram_tensor("u", [BC, DIM_IN], F32, kind="ExternalInput").ap()
    # wts: all constants in one tensor (one DMA): four 128x128 stationary
    # matrices host-rounded to e8m11 [Lhat^T | (D12/Lam)^T | Gu^T | Gw^T],
    # a fp32 identity (bit-exact in f32r), and xclam / c0 column vectors.
    wts = nc.dram_tensor("wts", [128, 642], F32R, kind="ExternalInput").ap()
    y = nc.dram_tensor("y", [BC, DIM_OUT], F32, kind="ExternalOutput").ap()

    # Batch rows interleaved so every partition's slab rows are contiguous
    # in DRAM (2KB descriptors): slab g, partition p holds rows
    # 512g + 4p + r (r = 0..3); SBUF free index = r*128 + f.
    u_r = u.rearrange("(g p r) f -> g p (r f)", p=128, r=4)
    y_r = y.rearrange("(g p r) f -> g p (r f)", p=128, r=4)

    with tile.TileContext(nc) as tc:
        with (
            tc.tile_pool(name="const", bufs=1) as cpool,
            tc.tile_pool(name="big", bufs=1) as bpool,
            tc.tile_pool(name="w", bufs=3) as wpool,
            tc.tile_pool(name="stage", bufs=4) as spool,
            tc.tile_pool(name="ps", bufs=8, space="PSUM") as ppool,
        ):
            ut = bpool.tile([128, BC], F32R, tag="ut")
            yt = bpool.tile([128, BC], F32, tag="yt")
            wts_t = cpool.tile([128, 642], F32R, tag="wts")

            # Constants go out first on the (otherwise idle) scalar HWDGE
            # queue; u slabs stream on both queues in parallel.
            nc.scalar.dma_start(wts_t[:], wts)
            ustage = [
                spool.tile([128, 512], F32, tag="ustage", name=f"ustage{g}")
                for g in range(NCH)
            ]
            for g in range(NCH):
                eng = nc.sync if g % 2 == 0 else nc.scalar
                eng.dma_start(ustage[g][:], u_r[g])

            lt = wts_t[:, 0:128]       # Lhat^T
            d12lt = wts_t[:, 128:256]  # (D12/Lam)^T
            gut = wts_t[:, 256:384]    # Gu^T
            gwt = wts_t[:, 384:512]    # Gw^T
            idt = wts_t[:, 512:640].bitcast(F32)  # fp32 identity
            xcl = wts_t[:, 640:641].bitcast(F32)  # xc/Lam  [128,1]
            c0 = wts_t[:, 641:642].bitcast(F32)   # C2 Einv F x0  [128,1]

            # ---- transpose u slabs to feature-major Ut (f32r) ----
            for g in range(NCH):
                pst = ppool.tile([128, 512], F32, tag="ps")
                for k in range(4):
                    ksl = slice(k * 128, (k + 1) * 128)
                    nc.tensor.transpose(pst[:, ksl], ustage[g][:, ksl], idt)
                sl = slice(g * 512, (g + 1) * 512)
                nc.vector.tensor_copy(ut[:, sl], pst[:])

            # ---- seed: W1 = tanh(D12L@Ut + xclam) ----
            w_cur = [None] * NCH
            for n in range(NCH):
                sl = slice(n * 512, (n + 1) * 512)
                ps = ppool.tile([128, 512], F32, tag="ps")
                nc.tensor.matmul(ps[:], d12lt, ut[:, sl], start=True, stop=True)
                wt = wpool.tile([128, 512], F32R, tag=f"w{n}")
                nc.scalar.activation(wt[:], ps[:], TANH, bias=xcl)
                w_cur[n] = wt

            # ---- Jacobi passes: W <- tanh(Lhat@W + D12L@Ut + xclam) ----
            # The constant UD term is recomputed by a second accumulating
            # matmul (same PE cost as adding a stored UDb, but no DVE add
            # and no extra SBUF tile); xclam rides the ACT bias.
            for _m in range(N_PASS):
                for n in range(NCH):
                    sl = slice(n * 512, (n + 1) * 512)
                    ps = ppool.tile([128, 512], F32, tag="ps")
                    nc.tensor.matmul(
                        ps[:], lt, w_cur[n][:], start=True, stop=False
                    )
                    nc.tensor.matmul(
                        ps[:], d12lt, ut[:, sl], start=False, stop=True
                    )
                    wt = wpool.tile([128, 512], F32R, tag=f"w{n}")
                    nc.scalar.activation(wt[:], ps[:], TANH, bias=xcl)
                    w_cur[n] = wt

            # ---- output: Yt = Gu@Ut + Gw@W + c0 ----
            for n in range(NCH):
                sl = slice(n * 512, (n + 1) * 512)
                ps = ppool.tile([128, 512], F32, tag="ps")
                nc.tensor.matmul(ps[:], gut, ut[:, sl], start=True, stop=False)
                nc.tensor.matmul(ps[:], gwt, w_cur[n][:], start=False, stop=True)
                nc.vector.tensor_scalar_add(yt[:, sl], ps[:], c0)

            # ---- transpose back to batch-major and store ----
            for g in range(NCH):
                pst = ppool.tile([128, 512], F32, tag="ps")
                for k in range(4):
                    ksl = slice(k * 128, (k + 1) * 128)
                    csl = slice(g * 512 + k * 128, g * 512 + (k + 1) * 128)
                    nc.tensor.transpose(pst[:, ksl], yt[:, csl], idt)
                ostage = spool.tile([128, 512], F32, tag="ostage")
                nc.vector.tensor_copy(ostage[:], pst[:])
                eng = nc.sync if g % 2 == 0 else nc.scalar
                eng.dma_start(y_r[g], ostage[:])
    nc.compile()
    return nc


def _derive_host_params(X, Y, B2, C2, D21, D22, D12, x0):
    """Fold the contractive parameterization into kernel constants (fp32,
    mirroring the reference's fp32 op order as closely as practical)."""
    f = np.float32
    X = np.ascontiguousarray(X, f)
    H = (X.T @ X + EPS * np.eye(DIM_H, dtype=f)).astype(f)
    H11 = H[:DIM_X, :DIM_X]
    H21 = H[DIM_X:DIM_X + DIM_NL, :DIM_X]
    H22 = H[DIM_X:DIM_X + DIM_NL, DIM_X:DIM_X + DIM_NL]
    H31 = H[DIM_X + DIM_NL:, :DIM_X]
    H32 = H[DIM_X + DIM_NL:, DIM_X:DIM_X + DIM_NL]
    H33 = H[DIM_X + DIM_NL:, DIM_X + DIM_NL:]
    F = H31
    B1 = H32
    E = (0.5 * (H11 + ALPHA * H33 + Y - Y.T)).astype(f)
    Lam = (0.5 * np.diagonal(H22)).astype(f)
    D11 = (-np.tril(H22, k=-1)).astype(f)
    C1 = -H21

    Einv = np.linalg.inv(E).astype(f)
    x0v = np.asarray(x0, f)[0, 0, :]
    xc = (C1 @ x0v).astype(f)
    fx = (F @ x0v).astype(f)

    Lhat = (D11 / Lam[:, None]).astype(f)
    D12L = (np.asarray(D12, f) / Lam[:, None]).astype(f)
    CE = (np.asarray(C2, f) @ Einv).astype(f)
    Gu = (CE @ B2 + D22).astype(f)
    Gw = (CE @ B1 + D21).astype(f)
    xclam = (xc / Lam).astype(f)
    c0 = (CE @ fx).astype(f)

    wts = np.zeros((128, 642), f)
    wts[:, 0:128] = Lhat.T
    wts[:, 128:256] = D12L.T
    wts[:, 256:384] = Gu.T
    wts[:, 384:512] = Gw.T
    wts = _round_f32r(wts)
    wts[:, 512:640] = np.eye(128, dtype=f)
    wts[:, 640] = _round_f32r(xclam)
    wts[:, 641] = _round_f32r(c0)
    return wts


def _in_maps(u_in, X, Y, B2, C2, D21, D22, D12, x0):
    wts = _derive_host_params(X, Y, B2, C2, D21, D22, D12, x0)
    u = np.ascontiguousarray(np.asarray(u_in, np.float32).reshape(B, DIM_IN))
    return [
        {"u": u[i * BC:(i + 1) * BC], "wts": wts}
        for i in range(N_CORES)
    ]


def kernel(u_in, X, Y, B2, C2, D21, D22, D12, x0):
    in_maps = _in_maps(u_in, X, Y, B2, C2, D21, D22, D12, x0)
    if "nc" not in _BUILT:
        _BUILT["nc"] = _build_nc()
    nc = _BUILT["nc"]
    res = run_bass_kernel_spmd(nc, in_maps, core_ids=list(range(N_CORES)))
    out = np.concatenate([res.results[i]["y"] for i in range(N_CORES)], axis=0)
    return out.reshape(B, 1, DIM_OUT).astype(np.float32)
